# revision 1
# baseline (speedup 1.0000x reference)
"""Causal self-attention on Trainium2, tensor-parallel over heads across 8 NeuronCores.

Strategy (sharding_hint "tensor-parallel split the n_heads axis"):
  - Each core c owns heads {2c, 2c+1} == columns [128c, 128c+128) of Wq/Wk/Wv
    and rows [128c, 128c+128) of Wo.
  - Per core: QT/KT = (x @ W{q,k})^T in [feat, tok] layout (both heads stacked
    on the 128 partitions: h0 rows 0-63, h1 rows 64-127; score matmuls contract
    K=64 via partition-offset APs / PE quadrants).  V in [tok, feat] layout
    with an appended ones column per head (denominator trick, 65-wide blocks).
  - Scores computed transposed ([k, q]) so exp(scoresT) feeds the PV matmul
    directly; row 64 of the PV output is the softmax denominator.
  - Normalization via reciprocal + K=1 broadcast matmul, multiply -> attnoutT.
  - Partial out-projection y_c = attnout_c @ Wo_c; host sums the 8 partials
    and adds bo + bv @ Wo (V-bias is a rank-1 term, folded on host).
  - bq/bk are folded into the PSUM->SBUF evacuation (per-partition
    tensor_scalar add), so no bias matmuls run on the PE.

Schedule: x is loaded window-major (one DMA per batch x 512-token window, all
in flight from t=0; weights stream on the scalar-engine DGE in parallel), so
the first QKV matmul starts ~5us in.  A single global filler deque (batch-1
QKV chunks, rolling per-window normalize + out-proj thunks) is drained two
thunks per attention pair-group, keeping PE demand high so the HAM clock gate
stays at 8/8; the serial tail is just the last window's norms + 4 out-proj
tiles.  Matmul inputs are bf16 (PSUM fp32).
"""

import sys

if "/opt/trn_rl_repo" not in sys.path:
    sys.path.insert(0, "/opt/trn_rl_repo")

from contextlib import ExitStack

import ml_dtypes
import numpy as np

import concourse.bass as bass
import concourse.mybir as mybir
import concourse.tile as tile

F32 = mybir.dt.float32
BF = mybir.dt.bfloat16
NPBF = ml_dtypes.bfloat16
EXP = mybir.ActivationFunctionType.Exp
LN = mybir.ActivationFunctionType.Ln

P = 128  # partition tile
HD = 64  # head dim
HC = 2  # heads per core (HC*HD == P)
WIN = 512  # token window (one PSUM bank of fp32)
VB = HD + 1  # v block: [V(64) | ones]
VW = HC * VB  # v cols per token tile
N_WARM = 8  # PE warm-up matmuls (run under the first x-window DMA shadow)


def _legalize_waits(nc):
    """This walrus build encodes at most ONE semaphore wait per instruction
    (setupSyncWait raises "Too many sync wait commands" otherwise).  Tile
    freely emits 2+ waits, so excess waits are moved onto injected same-engine
    NoOps (one wait each) directly before the instruction."""
    nop_id = 0
    for fn in nc.m.functions:
        for blk in fn.blocks:
            out = []
            for inst in blk.instructions:
                if type(inst).__name__ != "InstNoOp":
                    si = inst.sync_info
                    waits = list(si.on_wait or []) if si is not None else []
                    if len(waits) > 1:
                        for w in waits[1:]:
                            nop = mybir.InstNoOp(
                                name=f"nopw-{nop_id}",
                                engine=inst.engine,
                                ins=[],
                                outs=[],
                                sync_info=mybir.SyncInfo(on_wait=[w], on_update=[]),
                            )
                            nop_id += 1
                            out.append(nop)
                        si.on_wait = waits[:1]
                out.append(inst)
            blk.instructions[:] = out


def build_nc(B, T, D, n_cores, legalize=True):
    nj = D // P  # contraction tiles for projections
    n_win = T // WIN  # q windows per batch
    n_qt = T // P  # token tiles per batch
    tpw = n_qt // n_win  # token tiles per window
    M = B * T

    nc = bass.Bass("TRN2", target_bir_lowering=False, debug=False, num_devices=n_cores)

    xt = nc.dram_tensor("xt", [D, M], BF, kind="ExternalInput").ap()
    wq = nc.dram_tensor("wq", [P, D], BF, kind="ExternalInput").ap()
    wk = nc.dram_tensor("wk", [P, D], BF, kind="ExternalInput").ap()
    wv = nc.dram_tensor("wv", [P, D], BF, kind="ExternalInput").ap()
    wo = nc.dram_tensor("wo", [P, D], BF, kind="ExternalInput").ap()
    bq = nc.dram_tensor("bq", [P, 1], F32, kind="ExternalInput").ap()
    bk = nc.dram_tensor("bk", [P, 1], F32, kind="ExternalInput").ap()
    msk = nc.dram_tensor("msk", [P, P], BF, kind="ExternalInput").ap()
    y = nc.dram_tensor("y", [M, D], BF, kind="ExternalOutput").ap()

    with tile.TileContext(nc) as tc, ExitStack() as ctx:
        const = ctx.enter_context(tc.tile_pool(name="const", bufs=1))
        xbp = ctx.enter_context(tc.tile_pool(name="xb", bufs=2))
        qkp = ctx.enter_context(tc.tile_pool(name="qk", bufs=2))
        vp = ctx.enter_context(tc.tile_pool(name="vaug", bufs=2))
        atp = ctx.enter_context(tc.tile_pool(name="attnT", bufs=8))
        aop = ctx.enter_context(tc.tile_pool(name="aoT", bufs=2))
        pvp = ctx.enter_context(tc.tile_pool(name="pvs", bufs=6))
        rcp = ctx.enter_context(tc.tile_pool(name="rc", bufs=8))
        yp = ctx.enter_context(tc.tile_pool(name="ysb", bufs=6))
        ps_sc = ctx.enter_context(tc.tile_pool(name="ps_sc", bufs=2, space="PSUM"))
        ps_pv = ctx.enter_context(tc.tile_pool(name="ps_pv", bufs=2, space="PSUM"))
        ps_proj = ctx.enter_context(tc.tile_pool(name="ps_proj", bufs=2, space="PSUM"))

        wq_s = const.tile([P, D], BF, tag="wq")
        wk_s = const.tile([P, D], BF, tag="wk")
        wv_s = const.tile([P, D], BF, tag="wv")
        wo_s = const.tile([P, D], BF, tag="wo")
        bq_s = const.tile([P, 1], F32, tag="bq")
        bk_s = const.tile([P, 1], F32, tag="bk")
        msk_s = const.tile([P, P], BF, tag="msk")
        ones_f = const.tile([1, HD], F32, tag="onesf")
        warm_s = const.tile([P, WIN], BF, tag="warm")
        nc.vector.memset(ones_f[:, :], 1.0)
        nc.vector.memset(warm_s[:, :], 1.0)

        # x arrives window-major: one DMA per (batch, window) covering all nj
        # feature tiles, so window w's projections unblock after ~1MB.
        xbs = {}
        for b in range(B):
            xbs[b] = xbp.tile([P, n_win * nj * WIN], BF, tag="xb", name=f"xb{b}")

        def x_dma(b, w):
            src = xt[:, b * T + w * WIN : b * T + (w + 1) * WIN].rearrange(
                "(j p) c -> p j c", p=P
            )
            dst = xbs[b][:, w * nj * WIN : (w + 1) * nj * WIN].rearrange(
                "p (j c) -> p j c", j=nj
            )
            nc.sync.dma_start(dst, src)

        # q/k weights + first x window first; everything else behind them.
        nc.scalar.dma_start(wq_s[:, :], wq[:, :])
        x_dma(0, 0)
        nc.scalar.dma_start(wk_s[:, :], wk[:, :])
        nc.scalar.dma_start(bq_s[:, :], bq[:, :])
        nc.scalar.dma_start(bk_s[:, :], bk[:, :])
        x_dma(0, 1)
        nc.scalar.dma_start(wv_s[:, :], wv[:, :])
        nc.scalar.dma_start(msk_s[:, :], msk[:, :])
        x_dma(0, 2)
        x_dma(0, 3)
        nc.scalar.dma_start(wo_s[:, :], wo[:, :])
        for w in range(n_win):
            x_dma(1, w)

        # PE warm-up under the first x DMA's shadow.
        psw = ps_proj.tile([P, WIN], F32, tag="proj")
        for i in range(N_WARM):
            nc.tensor.matmul(
                psw[:, :], warm_s[:, 0:P], warm_s[:, :], start=True, stop=True
            )

        st = {}

        def xsl(b, w, j, off=0, n=WIN):
            base = (w * nj + j) * WIN + off
            return xbs[b][:, base : base + n]

        def alloc_batch(b):
            st[b] = {
                "qt": qkp.tile([P, T], BF, tag="qt", name=f"qt{b}"),
                "kt": qkp.tile([P, T], BF, tag="kt", name=f"kt{b}"),
                "vaug": vp.tile([P, n_qt * VW], BF, tag="vaug", name=f"vaug{b}"),
                "aoT": aop.tile([P, T], BF, tag="aoT", name=f"aoT{b}"),
                "vready": False,
            }

        def proj_chunk(b, w, which):
            s = st[b]
            ws = w * WIN
            w_s, b_s, dst = (
                (wq_s, bq_s, s["qt"]) if which == "q" else (wk_s, bk_s, s["kt"])
            )
            psp = ps_proj.tile([P, WIN], F32, tag="proj", name=f"ps{which}{b}_{w}")
            for j in range(nj):
                nc.tensor.matmul(
                    psp[:, :],
                    w_s[:, j * P : (j + 1) * P],
                    xsl(b, w, j),
                    start=(j == 0),
                    stop=(j == nj - 1),
                )
            # evac both heads in one DVE op, bias folded in (per-partition)
            nc.vector.tensor_scalar_add(
                dst[:, ws : ws + WIN], psp[:, :], b_s[:, 0:1]
            )

        def v_tile(b, t):
            s = st[b]
            if not s["vready"]:
                va4 = s["vaug"].rearrange("p (t s c) -> p t s c", s=HC, c=VB)
                nc.vector.memset(va4[:, :, :, HD : HD + 1], 1.0)  # ones col
                s["vready"] = True
            w, sub = t // tpw, t % tpw
            psv = ps_proj.tile([P, P], F32, tag="proj", name=f"psv{b}_{t}")
            for j in range(nj):
                nc.tensor.matmul(
                    psv[:, :],
                    xsl(b, w, j, sub * P, P),
                    wv_s[:, j * P : (j + 1) * P],
                    start=(j == 0),
                    stop=(j == nj - 1),
                )
            va4 = s["vaug"].rearrange("p (t s c) -> p t s c", s=HC, c=VB)
            nc.vector.tensor_copy(
                va4[:, t, :, 0:HD], psv[:, :].rearrange("p (s c) -> p s c", s=HC)
            )

        def qkv_thunks(b, w):
            th = [
                lambda b=b, w=w: proj_chunk(b, w, "q"),
                lambda b=b, w=w: proj_chunk(b, w, "k"),
            ]
            for t in range(w * tpw, (w + 1) * tpw):
                th.append(lambda b=b, t=t: v_tile(b, t))
            return th

        def normalize(b, pvsb, rc, hp, ws):
            psb = ps_sc.tile([HD, WIN], F32, tag="sc", name=f"psb{b}_{hp}_{ws}")
            nc.tensor.matmul(psb[:, :], ones_f[:, :], rc[:, :], start=True, stop=True)
            nc.vector.tensor_mul(
                st[b]["aoT"][hp : hp + HD, ws : ws + WIN], pvsb[0:HD, :], psb[:, :]
            )

        def outproj_tile(b, t, tail=False):
            toff = b * T
            aoT = st[b]["aoT"]
            ysb = yp.tile([P, D], BF, tag="ysb", name=f"ysb{b}_{t}")
            for ui, u0 in enumerate(range(0, D, WIN)):
                psy = ps_proj.tile([P, WIN], F32, tag="proj", name=f"psy{b}_{t}_{ui}")
                nc.tensor.matmul(
                    psy[:, :],
                    aoT[:, t * P : (t + 1) * P],
                    wo_s[:, u0 : u0 + WIN],
                    start=True,
                    stop=True,
                )
                if tail or tail_mode[0]:
                    # attention is over: the Activation engine is idle, so
                    # evacuate there and keep DVE off the critical tail path
                    nc.scalar.copy(ysb[:, u0 : u0 + WIN], psy[:, :])
                else:
                    nc.vector.tensor_copy(ysb[:, u0 : u0 + WIN], psy[:, :])
            nc.sync.dma_start(y[toff + t * P : toff + (t + 1) * P, :], ysb[:, :])

        # Global filler deque of (deadline_key, thunk).  Deadline keys are
        # global window indices (b*n_win+w) for QKV thunks that MUST trace
        # before that attention window; soft thunks (norms/out-proj) use 99.
        dq = []
        deferred = []
        tail_mode = [False]

        def pop_fill(n=1):
            for _ in range(n):
                if dq:
                    dq.pop(0)[1]()

        def force_drain(gwi):
            while any(k <= gwi for k, _ in dq):
                dq.pop(0)[1]()

        def drain_to(target):
            while len(dq) > target:
                dq.pop(0)[1]()

        def attn_window(b, w):
            # Heads run in disjoint PE quadrants (h0 rows 0-63, h1 64-127 via
            # partition-offset K=64 matmuls).  k tiles two at a time: both
            # score chunks of a head land in one 2-bank PSUM tile, one exp per
            # pair; PV for pair p is traced after the scores of pair p+1 so
            # the PE never waits on the exp.  Two filler thunks are popped
            # after each PV flush to keep PE demand (and the HAM clock) up.
            s = st[b]
            qt_s, kt_s, vaug = s["qt"], s["kt"], s["vaug"]
            ws = w * WIN
            njt = (ws + WIN) // P  # causal k tiles for this window
            pspv = [
                ps_pv.tile([VB, WIN], F32, tag="pv", name=f"pspv{b}_{w}_{_h}")
                for _h in range(HC)
            ]

            def flush_pv(at, halves):
                for h in range(HC):
                    for j, off, N, qstart in halves[h]:
                        vb = t_vb = j * VW + h * VB
                        nc.tensor.matmul(
                            pspv[h][:, qstart - ws : WIN],
                            vaug[:, vb : vb + VB],
                            at[h][:, off : off + N],
                            start=(j == 0),
                            stop=(j == njt - 1),
                        )

            prev = None
            for j0 in range(0, njt, 2):
                pss = [
                    ps_sc.tile([P, 2 * WIN], F32, tag="sc", name=f"pss{_h}")
                    for _h in range(HC)
                ]
                if not dq:
                    # keep the HAM clock gate seeing PE activity: fill both
                    # about-to-be-overwritten score banks with dummy matmuls
                    for _h in range(HC):
                        nc.tensor.matmul(
                            pss[_h][:, 0:WIN], warm_s[:, 0:P], warm_s[:, :],
                            start=True, stop=True,
                        )
                        nc.tensor.matmul(
                            pss[_h][:, WIN : 2 * WIN], warm_s[:, 0:P],
                            warm_s[:, :], start=True, stop=True,
                        )
                at = [
                    atp.tile([P, 2 * WIN], BF, tag="at", name=f"at{_h}")
                    for _h in range(HC)
                ]
                halves = [[] for _ in range(HC)]
                off = [0] * HC
                for j in (j0, j0 + 1):
                    if j >= njt:
                        continue
                    qstart = max(ws, j * P)
                    N = ws + WIN - qstart
                    for h in range(HC):
                        o = off[h]
                        if o and o + N > WIN:
                            o = WIN  # don't straddle a PSUM bank
                        nc.tensor.matmul(
                            pss[h][:, o : o + N],
                            kt_s[h * HD : (h + 1) * HD, j * P : (j + 1) * P],
                            qt_s[h * HD : (h + 1) * HD, qstart : qstart + N],
                            start=True,
                            stop=True,
                        )
                        halves[h].append((j, o, N, qstart))
                        off[h] = o + N
                for h in range(HC):
                    width = halves[h][-1][1] + halves[h][-1][2]
                    nc.scalar.activation(at[h][:, 0:width], pss[h][:, 0:width], EXP)
                    for j, o, N, qstart in halves[h]:
                        if j * P >= ws:  # zero the upper triangle post-exp
                            nc.gpsimd.tensor_mul(
                                at[h][:, o : o + P], at[h][:, o : o + P],
                                msk_s[:, :],
                            )
                if prev is not None:
                    flush_pv(*prev)
                    pop_fill(1)
                prev = (at, halves)
            flush_pv(*prev)
            pop_fill(2)
            # denominators + reciprocal now (reciprocal on DVE so the
            # Activation engine stays dedicated to the exps that pace the
            # attention pipeline); normalize + out-proj become filler thunks
            # drained during the next window.
            norm_args = []
            for h in range(HC):
                pvsb = pvp.tile([VB, WIN], F32, tag="pvs", name=f"pvsb{h}")
                nc.vector.tensor_copy(pvsb[:, :], pspv[h][:, :])
                lg = rcp.tile([1, WIN], F32, tag="lg", name=f"lg{h}")
                nc.scalar.activation(lg[:, :], pspv[h][HD : HD + 1, :], LN)
                rc = rcp.tile([1, WIN], F32, tag="rc", name=f"rc{h}")
                nc.scalar.activation(rc[:, :], lg[:, :], EXP, scale=-1.0)
                norm_args.append((pvsb, rc, h * HD, ws))
            for args in norm_args:
                dq.append((99, lambda b=b, a=args: normalize(b, *a)))
            for t in range(w * tpw, (w + 1) * tpw):
                dq.append((99, lambda b=b, t=t: outproj_tile(b, t)))

        # ---- schedule ----
        # b0 QKV runs up front; b1's QKV is deferred into the filler stream
        # with per-window deadlines (window w's chunks must trace before
        # b1-attention window w) so the second half of the kernel keeps
        # enough PE demand to hold the HAM clock at 8/8.  Window-end drain
        # targets shed backlog in bursts without starving later windows.
        alloc_batch(0)
        alloc_batch(1)
        for w in range(n_win):
            for f in qkv_thunks(0, w):
                f()
        for w in (0, 1):
            dq.extend((n_win + w, f) for f in qkv_thunks(1, w))
        for b in range(B):
            for w in range(n_win):
                force_drain(b * n_win + w)
                attn_window(b, w)
                if (b, w) == (0, 2):
                    dq.extend((n_win + 2, f) for f in qkv_thunks(1, 2))
                if (b, w) == (0, 3):
                    dq.extend((n_win + 3, f) for f in qkv_thunks(1, 3))
        tail_mode[0] = True
        while dq:
            dq.pop(0)[1]()

    if legalize:
        _legalize_waits(nc)
    return nc


def make_in_maps(x, Wq, bq, Wk, bk, Wv, Wo, n_cores):
    x = np.asarray(x, dtype=np.float32)
    Bb, Tt, Dd = x.shape
    M = Bb * Tt
    xt = np.ascontiguousarray(x.reshape(M, Dd).T.astype(NPBF))
    mask = np.where(
        np.arange(P)[:, None] > np.arange(P)[None, :], 0.0, 1.0
    ).astype(NPBF)

    def wslice(W, c, scale=1.0):
        Wc = np.asarray(W, np.float32)[:, c * P : (c + 1) * P] * np.float32(scale)
        return np.ascontiguousarray(
            Wc.reshape(Dd // P, P, P).transpose(1, 0, 2).reshape(P, Dd).astype(NPBF)
        )

    qscale = 1.0 / np.sqrt(HD)
    in_maps = []
    for c in range(n_cores):
        cs = slice(c * P, (c + 1) * P)
        in_maps.append(
            {
                "xt": xt,
                "wq": wslice(Wq, c, qscale),
                "wk": wslice(Wk, c),
                "wv": wslice(Wv, c),
                "wo": np.ascontiguousarray(
                    np.asarray(Wo, np.float32)[cs, :].astype(NPBF)
                ),
                "bq": np.ascontiguousarray(
                    (np.asarray(bq, np.float32)[cs] * np.float32(qscale)).reshape(
                        P, 1
                    )
                ),
                "bk": np.ascontiguousarray(
                    np.asarray(bk, np.float32)[cs].reshape(P, 1)
                ),
                "msk": mask,
            }
        )
    return in_maps


_NC_CACHE = {}


def get_nc(B, T, D, n_cores):
    key = (B, T, D, n_cores)
    if key not in _NC_CACHE:
        _NC_CACHE[key] = build_nc(B, T, D, n_cores)
    return _NC_CACHE[key]


def kernel(**inputs):
    from concourse.bass_utils import run_bass_kernel_spmd

    x = np.asarray(inputs["x"], np.float32)
    Bb, Tt, Dd = x.shape
    n_cores = 8
    nc = get_nc(Bb, Tt, Dd, n_cores)
    in_maps = make_in_maps(
        x,
        inputs["Wq"],
        inputs["bq"],
        inputs["Wk"],
        inputs["bk"],
        inputs["Wv"],
        inputs["Wo"],
        n_cores,
    )
    res = run_bass_kernel_spmd(nc, in_maps, core_ids=list(range(n_cores)))
    y = np.zeros((Bb * Tt, Dd), dtype=np.float64)
    for r in res.results:
        y += r["y"].astype(np.float64)
    # V-bias is rank-1 through Wo; fold it (and bo) on the host.
    y += (
        np.asarray(inputs["bv"], np.float64) @ np.asarray(inputs["Wo"], np.float64)
        + np.asarray(inputs["bo"], np.float64)
    )[None, :]
    return y.reshape(Bb, Tt, Dd).astype(np.float32)

